# revision 4
# baseline (speedup 1.0000x reference)
"""Trainium2 Bass kernel: causal self-attention with RoPE.

Model (matches the reference nn.Module):
    B=4, T=2048, C=1024, H=16 heads, head_dim=64
    qkv = x @ W_attn + b_attn ; rope(q, k) ; causal softmax(q k^T / 8) @ v
    out = y @ W_proj + b_proj

Sharding over 8 NeuronCores: data parallel on batch (4) x tensor parallel on
heads (2 groups of 8). Each core computes its batch's 8 heads end to end and
a partial y @ W_proj over its 512 head-dims; the host sums the two partial
projections per batch and adds b_proj.

Everything on-chip is kept in "feature on partitions" (transposed) layout so
all matmuls contract over the partition dim with zero transposes:
  x^T [C, T] -> Q^T/K^T [512, T] (RoPE fused into psum eviction via a
  partition-swapped copy + two table multiplies), V [T, 512] natural with a
  ones column per head (softmax denominator falls out of the same matmul that
  accumulates att @ V).  Matmuls run as float32r (TF32-like) at full PE rate.
"""

import os
import sys
from contextlib import ExitStack

for _p in ("/opt/trn_rl_repo", "/root/.axon_site/_ro/trn_rl_repo"):
    if os.path.isdir(_p) and _p not in sys.path:
        sys.path.append(_p)

import numpy as np

import bass_rust
import concourse.bass as bass
import concourse.mybir as mybir
from concourse import tile
from concourse.bass_utils import run_bass_kernel_spmd

F32 = mybir.dt.float32
F32R = mybir.dt.float32r
Act = mybir.ActivationFunctionType

B, T, C = 4, 2048, 1024
H, HD = 16, 64
HL = 8          # heads per core
N_CORES = 8
ROPE_BASE = 10000.0

T8 = 256        # t slice width for the qkv phase
QB = 512        # query block for attention
KB = 128        # key block for attention
NKB = T // KB   # 16
NQG = T // QB   # 4


def split_excess_waits(nc, max_waits=1):
    """The walrus build in this container supports only one sync-wait command
    per instruction; hoist extra semaphore waits onto same-engine NoOps
    inserted immediately before the instruction (same engine timeline, so
    semantics are unchanged)."""
    ctr = 0
    for fn in nc.m.functions:
        for blk in fn.blocks:
            new_insts = []
            changed = False
            for inst in blk.instructions:
                si = inst.sync_info
                if si is not None:
                    waits = list(si.on_wait)
                    sem_waits = [w for w in waits if w.sync_type == "semaphore"]
                    other = [w for w in waits if w.sync_type != "semaphore"]
                    budget = max(0, max_waits - len(other))
                    if len(sem_waits) > budget:
                        keep = sem_waits[:budget]
                        extra = sem_waits[budget:]
                        step = max(1, max_waits)
                        for i in range(0, len(extra), step):
                            nop = bass_rust.InstNoOp(
                                name=f"WSPLIT-{ctr}", ins=[], outs=[])
                            ctr += 1
                            nop.engine = inst.engine
                            nop.sync_info = bass_rust.SyncInfo(
                                on_wait=extra[i:i + step], on_update=[])
                            new_insts.append(nop)
                        si.on_wait = other + keep
                        changed = True
                new_insts.append(inst)
            if changed:
                blk.instructions = new_insts


def build_nc(split=True):
    nc = bass.Bass("TRN2", target_bir_lowering=False, debug=False,
                   num_devices=N_CORES)

    xT_d = nc.dram_tensor("xT", [C, T], F32R, kind="ExternalInput")
    wq_d = nc.dram_tensor("wq", [C, 512], F32R, kind="ExternalInput")
    wk_d = nc.dram_tensor("wk", [C, 512], F32R, kind="ExternalInput")
    wv_d = nc.dram_tensor("wv", [C, 512], F32R, kind="ExternalInput")
    wp_d = nc.dram_tensor("wp", [512, C], F32R, kind="ExternalInput")
    bq_d = nc.dram_tensor("bq", [512], F32, kind="ExternalInput")
    bk_d = nc.dram_tensor("bk", [512], F32, kind="ExternalInput")
    bv_d = nc.dram_tensor("bvrep", [128, 512], F32, kind="ExternalInput")
    cos_d = nc.dram_tensor("cos128", [128, T], F32R, kind="ExternalInput")
    sin_d = nc.dram_tensor("sin128", [128, T], F32R, kind="ExternalInput")
    mask_d = nc.dram_tensor("masks", [4, 128, 2 * QB], F32R,
                            kind="ExternalInput")
    ones_d = nc.dram_tensor("ones128", [128, 128], F32R, kind="ExternalInput")
    outT_d = nc.dram_tensor("outT", [C, T], F32, kind="ExternalOutput")

    with tile.TileContext(nc) as tc, ExitStack() as ctx:
        const = ctx.enter_context(tc.tile_pool(name="const", bufs=1))
        persist = ctx.enter_context(tc.tile_pool(name="persist", bufs=1))

        cos_sb = const.tile([128, T], F32R, tag="cos", name="cos_sb")
        sin_sb = const.tile([128, T], F32R, tag="sin", name="sin_sb")
        ones_sb = const.tile([128, 128], F32R, tag="ones", name="ones_sb")
        bq_sb = const.tile([128, 4], F32, tag="bq", name="bq_sb")
        bk_sb = const.tile([128, 4], F32, tag="bk", name="bk_sb")
        bv_sb = const.tile([128, 512], F32, tag="bv", name="bv_sb")
        nc.sync.dma_start(cos_sb[:], cos_d[:])
        nc.sync.dma_start(sin_sb[:], sin_d[:])
        nc.sync.dma_start(ones_sb[:], ones_d[:])
        nc.sync.dma_start(bq_sb[:],
                          bq_d.rearrange("(m p) -> p m", p=128))
        nc.sync.dma_start(bk_sb[:],
                          bk_d.rearrange("(m p) -> p m", p=128))
        nc.sync.dma_start(bv_sb[:], bv_d[:])

        # Persistent activations: Q^T, K^T (4 tiles each: tile i = local
        # heads 2i, 2i+1 as 128 rows), V ([t % 128, (head, t//128) * 65]
        # with a ones column at offset 64 of each 65-block), y^T.
        qt_t = [persist.tile([128, T], F32R, tag=f"qt{i}", name=f"qt{i}")
                for i in range(4)]
        kt_t = [persist.tile([128, T], F32R, tag=f"kt{i}", name=f"kt{i}")
                for i in range(4)]
        v_sb = persist.tile([128, HL * NKB * 65], F32R, tag="v", name="v_sb")

        # ones columns of V (written once; evictions fill the data columns)
        nc.sync.dma_start(
            v_sb[:].rearrange("p (blk c) -> p blk c", c=65)[:, :, 64:65],
            ones_d[:].rearrange("p (b o) -> p b o", o=1),
        )

        # ---------------- Phase A: QKV projection + RoPE ----------------
        with ExitStack() as pa:
            wpool = pa.enter_context(tc.tile_pool(name="wpool", bufs=1))
            xt_pool = pa.enter_context(tc.tile_pool(name="xt_pool", bufs=2))
            stage_pool = pa.enter_context(tc.tile_pool(name="stage_pool",
                                                       bufs=3))
            aux_pool = pa.enter_context(tc.tile_pool(name="aux_pool", bufs=3))
            qkv_ps = pa.enter_context(
                tc.tile_pool(name="qkv_ps", bufs=4, space="PSUM"))

            wq_sb = wpool.tile([128, 8 * 512], F32R, tag="wq", name="wq_sb")
            wk_sb = wpool.tile([128, 8 * 512], F32R, tag="wk", name="wk_sb")
            wv_sb = wpool.tile([128, 8 * 512], F32R, tag="wv", name="wv_sb")
            nc.sync.dma_start(
                wq_sb[:].rearrange("p (cc m) -> p cc m", cc=8),
                wq_d.rearrange("(cc p) m -> p cc m", p=128))
            nc.sync.dma_start(
                wk_sb[:].rearrange("p (cc m) -> p cc m", cc=8),
                wk_d.rearrange("(cc p) m -> p cc m", p=128))
            nc.sync.dma_start(
                wv_sb[:].rearrange("p (cc m) -> p cc m", cc=8),
                wv_d.rearrange("(cc p) m -> p cc m", p=128))

            n_t8 = T // T8
            for t8 in range(n_t8):
                ts, te = t8 * T8, (t8 + 1) * T8
                xt = xt_pool.tile([128, 8 * T8], F32R, tag="xt",
                                  name=f"xt{t8}")
                nc.sync.dma_start(
                    xt[:].rearrange("p (cc t) -> p cc t", cc=8),
                    xT_d[:, ts:te].rearrange("(cc p) t -> p cc t", p=128))

                for wsb, bias_sb, dst_tiles in (
                        (wq_sb, bq_sb, qt_t), (wk_sb, bk_sb, kt_t)):
                    for mc in range(4):
                        ps = qkv_ps.tile([128, T8], F32, tag="qkv_ps",
                                         name=f"ps_{t8}_{mc}")
                        for cc in range(8):
                            nc.tensor.matmul(
                                ps[:],
                                lhsT=wsb[:, cc * 512 + mc * 128:
                                         cc * 512 + (mc + 1) * 128],
                                rhs=xt[:, cc * T8:(cc + 1) * T8],
                                start=(cc == 0), stop=(cc == 7))
                        stage = stage_pool.tile([128, T8], F32R, tag="stage",
                                                name=f"stage_{t8}_{mc}")
                        nc.scalar.activation(stage[:], ps[:], Act.Identity,
                                             bias=bias_sb[:, mc:mc + 1])
                        # partition half-swap within each 64-row head block
                        aux = aux_pool.tile([128, T8], F32R, tag="aux",
                                            name=f"aux_{t8}_{mc}")
                        nc.sync.dma_start(aux[0:32, :], stage[32:64, :])
                        nc.sync.dma_start(aux[32:64, :], stage[0:32, :])
                        nc.sync.dma_start(aux[64:96, :], stage[96:128, :])
                        nc.sync.dma_start(aux[96:128, :], stage[64:96, :])
                        dst = dst_tiles[mc][:, ts:te]
                        nc.vector.tensor_mul(dst, stage[:], cos_sb[:, ts:te])
                        nc.vector.tensor_mul(aux[:], aux[:], sin_sb[:, ts:te])
                        nc.vector.tensor_add(dst, dst, aux[:])

                for tbl in range(T8 // 128):
                    tb = (t8 * T8) // 128 + tbl
                    ps = qkv_ps.tile([128, 512], F32, tag="qkv_ps",
                                     name=f"psv_{t8}_{tbl}")
                    for cc in range(8):
                        nc.tensor.matmul(
                            ps[:],
                            lhsT=xt[:, cc * T8 + tbl * 128:
                                    cc * T8 + tbl * 128 + 128],
                            rhs=wv_sb[:, cc * 512:(cc + 1) * 512],
                            start=(cc == 0), stop=(cc == 7))
                    for h in range(HL):
                        off = (h * NKB + tb) * 65
                        nc.vector.tensor_add(
                            v_sb[:, off:off + 64],
                            ps[:, h * 64:(h + 1) * 64],
                            bv_sb[:, h * 64:(h + 1) * 64])

        # ---------------- Phase B + C scope ----------------
        with ExitStack() as pbc:
            ytpool = pbc.enter_context(tc.tile_pool(name="ytpool", bufs=1))
            yt_t = [ytpool.tile([128, T], F32R, tag=f"yt{i}", name=f"yt{i}")
                    for i in range(4)]

            # ---------------- Phase B: causal attention ----------------
            with ExitStack() as pb:
                mpool = pb.enter_context(tc.tile_pool(name="mpool", bufs=1))
                pt_pool = pb.enter_context(tc.tile_pool(name="pt_pool",
                                                        bufs=3))
                misc_pool = pb.enter_context(tc.tile_pool(name="misc_pool",
                                                          bufs=2))
                s_pool = pb.enter_context(
                    tc.tile_pool(name="s_pool", bufs=2, space="PSUM"))
                o_pool = pb.enter_context(
                    tc.tile_pool(name="o_pool", bufs=1, space="PSUM"))
                b_pool = pb.enter_context(
                    tc.tile_pool(name="b_pool", bufs=1, space="PSUM"))

                mask_sb = [mpool.tile([128, 2 * QB], F32R, tag=f"mask{r}",
                                      name=f"mask{r}") for r in range(4)]
                for r in range(4):
                    nc.sync.dma_start(mask_sb[r][:], mask_d[r])

                for g in range(NQG):
                    qs, qe = g * QB, (g + 1) * QB
                    for hp in range(4):
                        o_ps = [o_pool.tile([65, 512], F32, tag=f"o{hh}",
                                            name=f"o{hh}_{g}_{hp}")
                                for hh in range(2)]
                        nkb = 4 * g + 4
                        for kb in range(nkb):
                            s_ps = s_pool.tile([128, 2 * QB], F32, tag="s",
                                               name=f"s_{g}_{hp}_{kb}")
                            for hh in range(2):
                                nc.tensor.matmul(
                                    s_ps[:, hh * QB:(hh + 1) * QB],
                                    lhsT=kt_t[hp][hh * 64:(hh + 1) * 64,
                                                  kb * KB:(kb + 1) * KB],
                                    rhs=qt_t[hp][hh * 64:(hh + 1) * 64,
                                                 qs:qe],
                                    start=True, stop=True,
                                    tile_position=(hh * 64, 0))
                            pt = pt_pool.tile([128, 2 * QB], F32R, tag="pt",
                                              name=f"pt_{g}_{hp}_{kb}")
                            nc.scalar.activation(pt[:], s_ps[:], Act.Exp,
                                                 scale=0.125)
                            if kb >= 4 * g:
                                nc.vector.tensor_mul(pt[:], pt[:],
                                                     mask_sb[kb - 4 * g][:])
                            for hh in range(2):
                                h = hp * 2 + hh
                                off = (h * NKB + kb) * 65
                                nc.tensor.matmul(
                                    o_ps[hh][:],
                                    lhsT=v_sb[:, off:off + 65],
                                    rhs=pt[:, hh * QB:(hh + 1) * QB],
                                    start=(kb == 0), stop=(kb == nkb - 1))
                        for hh in range(2):
                            recip = misc_pool.tile([128, 512], F32R,
                                                   tag="recip",
                                                   name=f"rc_{g}_{hp}_{hh}")
                            with nc.allow_low_precision(
                                    reason="fp32r softmax denominators"):
                                nc.vector.reciprocal(recip[64:65, :],
                                                     o_ps[hh][64:65, :])
                            b_ps = b_pool.tile([64, 512], F32, tag="b",
                                               name=f"b_{g}_{hp}_{hh}")
                            nc.tensor.matmul(
                                b_ps[:], lhsT=ones_sb[64:65, 0:64],
                                rhs=recip[64:65, :], start=True, stop=True)
                            o_sb = misc_pool.tile([64, 512], F32, tag="osb",
                                                  name=f"ob_{g}_{hp}_{hh}")
                            nc.scalar.copy(o_sb[:], o_ps[hh][0:64, :])
                            nc.vector.tensor_mul(
                                yt_t[hp][hh * 64:(hh + 1) * 64, qs:qe],
                                o_sb[:], b_ps[:])

            # ---------------- Phase C: output projection ----------------
            with ExitStack() as pc:
                wp_pool = pc.enter_context(tc.tile_pool(name="wp_pool",
                                                        bufs=1))
                out_pool = pc.enter_context(tc.tile_pool(name="out_pool",
                                                         bufs=3))
                proj_ps = pc.enter_context(
                    tc.tile_pool(name="proj_ps", bufs=3, space="PSUM"))

                wp_sb = wp_pool.tile([128, 4 * C], F32R, tag="wp",
                                     name="wp_sb")
                nc.sync.dma_start(
                    wp_sb[:].rearrange("p (cc co) -> p cc co", cc=4),
                    wp_d.rearrange("(cc p) co -> p cc co", p=128))

                for co in range(8):
                    for tg in range(NQG):
                        ps = proj_ps.tile([128, 512], F32, tag="pps",
                                          name=f"pps_{co}_{tg}")
                        for cc in range(4):
                            nc.tensor.matmul(
                                ps[:],
                                lhsT=wp_sb[:, cc * C + co * 128:
                                           cc * C + (co + 1) * 128],
                                rhs=yt_t[cc][:, tg * 512:(tg + 1) * 512],
                                start=(cc == 0), stop=(cc == 3))
                        osb = out_pool.tile([128, 512], F32, tag="out",
                                            name=f"out_{co}_{tg}")
                        nc.scalar.copy(osb[:], ps[:])
                        nc.sync.dma_start(
                            outT_d[co * 128:(co + 1) * 128,
                                   tg * 512:(tg + 1) * 512],
                            osb[:])

    if split:
        split_excess_waits(nc)
    return nc


_NC = None


def _get_nc():
    global _NC
    if _NC is None:
        _NC = build_nc()
    return _NC


def _rope_tables_128():
    rot = HD // 2  # 32
    inv_freq = 1.0 / (ROPE_BASE ** (np.arange(0, rot, 2, dtype=np.float32)
                                    / np.float32(rot)))
    pos = np.arange(T, dtype=np.float32)
    freqs = np.outer(pos, inv_freq).astype(np.float32)   # [T, 16]
    emb = np.concatenate([freqs, freqs], axis=-1)        # [T, 32]
    cosT = np.cos(emb).astype(np.float32).T              # [32, T]
    sinT = np.sin(emb).astype(np.float32).T
    cos128 = np.ascontiguousarray(np.tile(cosT, (4, 1)))
    sgn = np.ones((128, 1), np.float32)
    sgn[0:32] = -1.0
    sgn[64:96] = -1.0
    sin128 = np.ascontiguousarray(np.tile(sinT, (4, 1)) * sgn)
    return cos128, sin128


def _masks():
    kp = np.arange(128, dtype=np.int64)[:, None]
    qf = np.arange(QB, dtype=np.int64)[None, :]
    out = np.empty((4, 128, 2 * QB), np.float32)
    for r in range(4):
        base = ((r * KB + kp) <= qf).astype(np.float32)
        out[r] = np.concatenate([base, base], axis=1)
    return out


def _in_maps(x, W_attn, b_attn, W_proj):
    cos128, sin128 = _rope_tables_128()
    masks = _masks()
    ones = np.ones((128, 128), np.float32)
    maps = []
    for c in range(N_CORES):
        b, hg = c // 2, c % 2
        sl = slice(hg * 512, (hg + 1) * 512)
        maps.append({
            "xT": np.ascontiguousarray(x[b].T),
            "wq": np.ascontiguousarray(W_attn[:, 0 * C:1 * C][:, sl]),
            "wk": np.ascontiguousarray(W_attn[:, 1 * C:2 * C][:, sl]),
            "wv": np.ascontiguousarray(W_attn[:, 2 * C:3 * C][:, sl]),
            "wp": np.ascontiguousarray(W_proj[sl, :]),
            "bq": np.ascontiguousarray(b_attn[0 * C:1 * C][sl]),
            "bk": np.ascontiguousarray(b_attn[1 * C:2 * C][sl]),
            "bvrep": np.ascontiguousarray(
                np.broadcast_to(b_attn[2 * C:3 * C][sl], (128, 512))),
            "cos128": cos128,
            "sin128": sin128,
            "masks": masks,
            "ones128": ones,
        })
    return maps


def kernel(x, W_attn, b_attn, W_proj, b_proj):
    x = np.asarray(x, dtype=np.float32)
    W_attn = np.asarray(W_attn, dtype=np.float32)
    b_attn = np.asarray(b_attn, dtype=np.float32)
    W_proj = np.asarray(W_proj, dtype=np.float32)
    b_proj = np.asarray(b_proj, dtype=np.float32)

    nc = _get_nc()
    maps = _in_maps(x, W_attn, b_attn, W_proj)
    res = run_bass_kernel_spmd(nc, maps, list(range(N_CORES)))

    out = np.empty((B, T, C), np.float32)
    for b in range(B):
        acc = res.results[2 * b]["outT"] + res.results[2 * b + 1]["outT"]
        out[b] = acc.T + b_proj[None, :]
    return out


# revision 7
# speedup vs baseline: 1.0629x; 1.0629x over previous
"""Trainium2 Bass kernel: causal self-attention with RoPE.

Model (matches the reference nn.Module):
    B=4, T=2048, C=1024, H=16 heads, head_dim=64
    qkv = x @ W_attn + b_attn ; rope(q, k) ; causal softmax(q k^T / 8) @ v
    out = y @ W_proj + b_proj

Sharding over 8 NeuronCores: data parallel on batch (4) x tensor parallel on
heads (2 groups of 8). Each core computes its batch's 8 heads end to end and
a partial y @ W_proj over its 512 head-dims; the host sums the two partial
projections per batch and adds b_proj.

Everything on-chip stays in "feature on partitions" (transposed) layout so
every matmul contracts over the partition dim with zero transposes:
  x^T [C,T] -> K^T [512,T] resident / Q^T per 512-query stripe (RoPE fused
  into the psum eviction via a partition-swapped SBUF->SBUF DMA + two table
  multiplies), V [T,512] natural with a ones column per head (the softmax
  denominator falls out of the same matmul that accumulates att @ V).
  Matmuls run as float32r (TF32-like) at full PE rate.

The program is emitted stripe-interleaved (QKV for 512 t-columns ->
attention for that query stripe -> output projection for those columns) with
every tile pool open for the whole kernel, so the Tile scheduler can overlap
the PE-heavy projection phases with the ACT-heavy softmax phase.
"""

import os
import sys
from contextlib import ExitStack

for _p in ("/opt/trn_rl_repo", "/root/.axon_site/_ro/trn_rl_repo"):
    if os.path.isdir(_p) and _p not in sys.path:
        sys.path.append(_p)

import numpy as np

import bass_rust
import concourse.bass as bass
import concourse.mybir as mybir
from concourse import tile
from concourse.bass_utils import run_bass_kernel_spmd

F32 = mybir.dt.float32
F32R = mybir.dt.float32r
Act = mybir.ActivationFunctionType

B, T, C = 4, 2048, 1024
H, HD = 16, 64
HL = 8          # heads per core
N_CORES = 8
ROPE_BASE = 10000.0

T8 = 256        # t slice width for the qkv phase
QB = 512        # query stripe width
KB = 128        # key block for attention
NKB = T // KB   # 16
NQG = T // QB   # 4


def split_excess_waits(nc, max_waits=1):
    """The walrus build in this container supports only one sync-wait command
    per instruction; hoist extra semaphore waits onto same-engine NoOps
    inserted immediately before the instruction (same engine timeline, so
    semantics are unchanged)."""
    ctr = 0
    for fn in nc.m.functions:
        for blk in fn.blocks:
            new_insts = []
            changed = False
            for inst in blk.instructions:
                si = inst.sync_info
                if si is not None:
                    waits = list(si.on_wait)
                    sem_waits = [w for w in waits if w.sync_type == "semaphore"]
                    other = [w for w in waits if w.sync_type != "semaphore"]
                    budget = max(0, max_waits - len(other))
                    if len(sem_waits) > budget:
                        keep = sem_waits[:budget]
                        extra = sem_waits[budget:]
                        step = max(1, max_waits)
                        for i in range(0, len(extra), step):
                            nop = bass_rust.InstNoOp(
                                name=f"WSPLIT-{ctr}", ins=[], outs=[])
                            ctr += 1
                            nop.engine = inst.engine
                            nop.sync_info = bass_rust.SyncInfo(
                                on_wait=extra[i:i + step], on_update=[])
                            new_insts.append(nop)
                        si.on_wait = other + keep
                        changed = True
                new_insts.append(inst)
            if changed:
                blk.instructions = new_insts


def build_nc(split=True):
    nc = bass.Bass("TRN2", target_bir_lowering=False, debug=False,
                   num_devices=N_CORES)

    xT_d = nc.dram_tensor("xT", [C, T], F32R, kind="ExternalInput")
    wq_d = nc.dram_tensor("wq", [C, 512], F32R, kind="ExternalInput")
    wk_d = nc.dram_tensor("wk", [C, 512], F32R, kind="ExternalInput")
    wv_d = nc.dram_tensor("wv", [C, 512], F32R, kind="ExternalInput")
    wp_d = nc.dram_tensor("wp", [512, C], F32R, kind="ExternalInput")
    bq_d = nc.dram_tensor("bq", [512], F32, kind="ExternalInput")
    bk_d = nc.dram_tensor("bk", [512], F32, kind="ExternalInput")
    bv_d = nc.dram_tensor("bvrep", [128, 512], F32, kind="ExternalInput")
    cos_d = nc.dram_tensor("cos128", [128, T], F32R, kind="ExternalInput")
    sin_d = nc.dram_tensor("sin128", [128, T], F32R, kind="ExternalInput")
    mask_d = nc.dram_tensor("masks", [4, 128, QB], F32R, kind="ExternalInput")
    ones_d = nc.dram_tensor("ones128", [128, 128], F32R, kind="ExternalInput")
    outT_d = nc.dram_tensor("outT", [C, T], F32, kind="ExternalOutput")

    with tile.TileContext(nc) as tc, ExitStack() as ctx:
        const = ctx.enter_context(tc.tile_pool(name="const", bufs=1))
        persist = ctx.enter_context(tc.tile_pool(name="persist", bufs=1))
        wres = ctx.enter_context(tc.tile_pool(name="wres", bufs=1))
        cs_pool = ctx.enter_context(tc.tile_pool(name="cs_pool", bufs=2))
        xt_pool = ctx.enter_context(tc.tile_pool(name="xt_pool", bufs=2))
        qts_pool = ctx.enter_context(tc.tile_pool(name="qts_pool", bufs=2))
        aux_pool = ctx.enter_context(tc.tile_pool(name="aux_pool", bufs=3))
        pt_pool = ctx.enter_context(tc.tile_pool(name="pt_pool", bufs=2))
        misc_pool = ctx.enter_context(tc.tile_pool(name="misc_pool", bufs=2))
        yt_pool = ctx.enter_context(tc.tile_pool(name="yt_pool", bufs=1))
        out_pool = ctx.enter_context(tc.tile_pool(name="out_pool", bufs=2))
        ps_pool = ctx.enter_context(
            tc.tile_pool(name="ps_pool", bufs=2, space="PSUM"))
        o_pool = ctx.enter_context(
            tc.tile_pool(name="o_pool", bufs=1, space="PSUM"))

        # ---- constants ----
        ones_sb = const.tile([128, 128], F32R, tag="ones", name="ones_sb")
        bq_sb = const.tile([128, 4], F32, tag="bq", name="bq_sb")
        bk_sb = const.tile([128, 4], F32, tag="bk", name="bk_sb")
        bv_sb = const.tile([128, 512], F32, tag="bv", name="bv_sb")
        mask_sb = [const.tile([128, QB], F32R, tag=f"mask{r}",
                              name=f"mask{r}") for r in range(4)]
        nc.sync.dma_start(ones_sb[:], ones_d[:])
        nc.sync.dma_start(bq_sb[:], bq_d.rearrange("(m p) -> p m", p=128))
        nc.sync.dma_start(bk_sb[:], bk_d.rearrange("(m p) -> p m", p=128))
        nc.sync.dma_start(bv_sb[:], bv_d[:])

        # ---- persistent activations ----
        kt_t = [persist.tile([128, T], F32R, tag=f"kt{i}", name=f"kt{i}")
                for i in range(4)]
        v_sb = persist.tile([128, HL * NKB * 65], F32R, tag="v", name="v_sb")
        nc.sync.dma_start(
            v_sb[:].rearrange("p (blk c) -> p blk c", c=65)[:, :, 64:65],
            ones_d[:].rearrange("p (b o) -> p b o", o=1),
        )

        # ---- resident weights (split per 128-row chunk for fine deps) ----
        wq_sb = wres.tile([128, 8 * 512], F32R, tag="wq", name="wq_sb")
        wk_sb = wres.tile([128, 8 * 512], F32R, tag="wk", name="wk_sb")
        wv_sb = wres.tile([128, 8 * 512], F32R, tag="wv", name="wv_sb")
        wp_sb = wres.tile([128, 4 * C], F32R, tag="wp", name="wp_sb")
        for cc in range(8):
            nc.sync.dma_start(wq_sb[:, cc * 512:(cc + 1) * 512],
                              wq_d[cc * 128:(cc + 1) * 128, :])
            nc.sync.dma_start(wk_sb[:, cc * 512:(cc + 1) * 512],
                              wk_d[cc * 128:(cc + 1) * 128, :])
            nc.sync.dma_start(wv_sb[:, cc * 512:(cc + 1) * 512],
                              wv_d[cc * 128:(cc + 1) * 128, :])
        for cc in range(4):
            nc.sync.dma_start(wp_sb[:, cc * C:(cc + 1) * C],
                              wp_d[cc * 128:(cc + 1) * 128, :])
        for r in range(4):
            nc.sync.dma_start(mask_sb[r][:], mask_d[r])

        for g in range(NQG):
            # ------- QKV + RoPE for t columns [g*512, (g+1)*512) -------
            qts = [qts_pool.tile([128, QB], F32R, tag=f"qts{mc}",
                                 name=f"qts{mc}_{g}") for mc in range(4)]
            for t8l in range(2):
                t8 = 2 * g + t8l
                ts, te = t8 * T8, (t8 + 1) * T8
                xt = xt_pool.tile([128, 8 * T8], F32R, tag="xt",
                                  name=f"xt{t8}")
                xv = xt[:].rearrange("p (cc t) -> p cc t", cc=8)
                for half in range(2):
                    nc.sync.dma_start(
                        xv[:, half * 4:(half + 1) * 4],
                        xT_d[512 * half:512 * (half + 1), ts:te]
                        .rearrange("(cc p) t -> p cc t", p=128))
                cosS = cs_pool.tile([128, T8], F32R, tag="cosS",
                                    name=f"cosS{t8}")
                sinS = cs_pool.tile([128, T8], F32R, tag="sinS",
                                    name=f"sinS{t8}")
                nc.sync.dma_start(cosS[:], cos_d[:, ts:te])
                nc.sync.dma_start(sinS[:], sin_d[:, ts:te])

                for wsb, bias_sb, is_q in ((wq_sb, bq_sb, True),
                                           (wk_sb, bk_sb, False)):
                    for mc in range(4):
                        ps = ps_pool.tile([128, T8], F32, tag="qkv",
                                          name=f"ps{t8}_{mc}_{int(is_q)}")
                        for cc in range(8):
                            nc.tensor.matmul(
                                ps[:],
                                lhsT=wsb[:, cc * 512 + mc * 128:
                                         cc * 512 + (mc + 1) * 128],
                                rhs=xt[:, cc * T8:(cc + 1) * T8],
                                start=(cc == 0), stop=(cc == 7))
                        if is_q:
                            dst = qts[mc][:, t8l * T8:(t8l + 1) * T8]
                        else:
                            dst = kt_t[mc][:, ts:te]
                        nc.vector.tensor_scalar_add(dst, ps[:],
                                                    bias_sb[:, mc:mc + 1])
                        aux = aux_pool.tile([128, T8], F32R, tag="aux",
                                            name=f"aux{t8}_{mc}_{int(is_q)}")
                        nc.sync.dma_start(aux[0:32, :], dst[32:64, :])
                        nc.sync.dma_start(aux[32:64, :], dst[0:32, :])
                        nc.sync.dma_start(aux[64:96, :], dst[96:128, :])
                        nc.sync.dma_start(aux[96:128, :], dst[64:96, :])
                        nc.vector.tensor_mul(dst, dst, cosS[:])
                        nc.gpsimd.tensor_mul(aux[:], aux[:], sinS[:])
                        nc.vector.tensor_add(dst, dst, aux[:])

                for tbl in range(T8 // 128):
                    tb = t8 * (T8 // 128) + tbl
                    ps = ps_pool.tile([128, 512], F32, tag="qkv",
                                      name=f"psv{t8}_{tbl}")
                    for cc in range(8):
                        nc.tensor.matmul(
                            ps[:],
                            lhsT=xt[:, cc * T8 + tbl * 128:
                                    cc * T8 + tbl * 128 + 128],
                            rhs=wv_sb[:, cc * 512:(cc + 1) * 512],
                            start=(cc == 0), stop=(cc == 7))
                    for h in range(HL):
                        off = (h * NKB + tb) * 65
                        nc.vector.tensor_add(
                            v_sb[:, off:off + 64],
                            ps[:, h * 64:(h + 1) * 64],
                            bv_sb[:, h * 64:(h + 1) * 64])

            # ------- causal attention for query stripe g -------
            for hp in range(4):
                o_ps = [o_pool.tile([65, 512], F32, tag=f"o{hh}",
                                    name=f"o{hh}_{g}_{hp}")
                        for hh in range(2)]
                nkb = 4 * g + 4
                for kb in range(nkb):
                    s_ps = ps_pool.tile([128, 2 * QB], F32, tag="s",
                                        name=f"s_{g}_{hp}_{kb}")
                    for hh in range(2):
                        nc.tensor.matmul(
                            s_ps[:, hh * QB:(hh + 1) * QB],
                            lhsT=kt_t[hp][hh * 64:(hh + 1) * 64,
                                          kb * KB:(kb + 1) * KB],
                            rhs=qts[hp][hh * 64:(hh + 1) * 64, :],
                            start=True, stop=True,
                            tile_position=(hh * 64, 0))
                    pt = pt_pool.tile([128, 2 * QB], F32R, tag="pt",
                                      name=f"pt_{g}_{hp}_{kb}")
                    nc.scalar.activation(pt[:], s_ps[:], Act.Exp, scale=0.125)
                    if kb >= 4 * g:
                        r = kb - 4 * g
                        nc.vector.tensor_mul(pt[:, 0:QB], pt[:, 0:QB],
                                             mask_sb[r][:])
                        nc.gpsimd.tensor_mul(pt[:, QB:2 * QB], pt[:, QB:2 * QB],
                                             mask_sb[r][:])
                    for hh in range(2):
                        h = hp * 2 + hh
                        off = (h * NKB + kb) * 65
                        nc.tensor.matmul(
                            o_ps[hh][:],
                            lhsT=v_sb[:, off:off + 65],
                            rhs=pt[:, hh * QB:(hh + 1) * QB],
                            start=(kb == 0), stop=(kb == nkb - 1))
                if hp == 0:
                    yts = [yt_pool.tile([128, QB], F32R, tag=f"yt{i}",
                                        name=f"yt{i}_{g}") for i in range(4)]
                for hh in range(2):
                    recip = misc_pool.tile([65, 512], F32R, tag="recip",
                                           name=f"rc_{g}_{hp}_{hh}")
                    with nc.allow_low_precision(
                            reason="fp32r softmax denominators"):
                        nc.vector.reciprocal(recip[64:65, :],
                                             o_ps[hh][64:65, :])
                    b_ps = ps_pool.tile([64, 512], F32, tag="qkv",
                                        name=f"b_{g}_{hp}_{hh}")
                    nc.tensor.matmul(b_ps[:], lhsT=ones_sb[64:65, 0:64],
                                     rhs=recip[64:65, :],
                                     start=True, stop=True)
                    o_sb = misc_pool.tile([64, 512], F32, tag="osb",
                                          name=f"ob_{g}_{hp}_{hh}")
                    nc.scalar.copy(o_sb[:], o_ps[hh][0:64, :])
                    nc.vector.tensor_mul(yts[hp][hh * 64:(hh + 1) * 64, :],
                                         o_sb[:], b_ps[:])

            # ------- output projection for t columns of stripe g -------
            for co in range(8):
                ps = ps_pool.tile([128, 512], F32, tag="qkv",
                                  name=f"pps_{g}_{co}")
                for cc in range(4):
                    nc.tensor.matmul(
                        ps[:],
                        lhsT=wp_sb[:, cc * C + co * 128:
                                   cc * C + (co + 1) * 128],
                        rhs=yts[cc][:],
                        start=(cc == 0), stop=(cc == 3))
                osb = out_pool.tile([128, 512], F32, tag="out",
                                    name=f"out_{g}_{co}")
                nc.scalar.copy(osb[:], ps[:])
                nc.sync.dma_start(
                    outT_d[co * 128:(co + 1) * 128, g * QB:(g + 1) * QB],
                    osb[:])

    if split:
        split_excess_waits(nc)
    return nc


_NC = None


def _get_nc():
    global _NC
    if _NC is None:
        _NC = build_nc()
    return _NC


def _rope_tables_128():
    rot = HD // 2  # 32
    inv_freq = 1.0 / (ROPE_BASE ** (np.arange(0, rot, 2, dtype=np.float32)
                                    / np.float32(rot)))
    pos = np.arange(T, dtype=np.float32)
    freqs = np.outer(pos, inv_freq).astype(np.float32)   # [T, 16]
    emb = np.concatenate([freqs, freqs], axis=-1)        # [T, 32]
    cosT = np.cos(emb).astype(np.float32).T              # [32, T]
    sinT = np.sin(emb).astype(np.float32).T
    cos128 = np.ascontiguousarray(np.tile(cosT, (4, 1)))
    sgn = np.ones((128, 1), np.float32)
    sgn[0:32] = -1.0
    sgn[64:96] = -1.0
    sin128 = np.ascontiguousarray(np.tile(sinT, (4, 1)) * sgn)
    return cos128, sin128


def _masks():
    kp = np.arange(128, dtype=np.int64)[:, None]
    qf = np.arange(QB, dtype=np.int64)[None, :]
    out = np.empty((4, 128, QB), np.float32)
    for r in range(4):
        out[r] = ((r * KB + kp) <= qf).astype(np.float32)
    return out


def _in_maps(x, W_attn, b_attn, W_proj):
    cos128, sin128 = _rope_tables_128()
    masks = _masks()
    ones = np.ones((128, 128), np.float32)
    maps = []
    for c in range(N_CORES):
        b, hg = c // 2, c % 2
        sl = slice(hg * 512, (hg + 1) * 512)
        maps.append({
            "xT": np.ascontiguousarray(x[b].T),
            "wq": np.ascontiguousarray(W_attn[:, 0 * C:1 * C][:, sl]),
            "wk": np.ascontiguousarray(W_attn[:, 1 * C:2 * C][:, sl]),
            "wv": np.ascontiguousarray(W_attn[:, 2 * C:3 * C][:, sl]),
            "wp": np.ascontiguousarray(W_proj[sl, :]),
            "bq": np.ascontiguousarray(b_attn[0 * C:1 * C][sl]),
            "bk": np.ascontiguousarray(b_attn[1 * C:2 * C][sl]),
            "bvrep": np.ascontiguousarray(
                np.broadcast_to(b_attn[2 * C:3 * C][sl], (128, 512))),
            "cos128": cos128,
            "sin128": sin128,
            "masks": masks,
            "ones128": ones,
        })
    return maps


def kernel(x, W_attn, b_attn, W_proj, b_proj):
    x = np.asarray(x, dtype=np.float32)
    W_attn = np.asarray(W_attn, dtype=np.float32)
    b_attn = np.asarray(b_attn, dtype=np.float32)
    W_proj = np.asarray(W_proj, dtype=np.float32)
    b_proj = np.asarray(b_proj, dtype=np.float32)

    nc = _get_nc()
    maps = _in_maps(x, W_attn, b_attn, W_proj)
    res = run_bass_kernel_spmd(nc, maps, list(range(N_CORES)))

    out = np.empty((B, T, C), np.float32)
    for b in range(B):
        acc = res.results[2 * b]["outT"] + res.results[2 * b + 1]["outT"]
        out[b] = acc.T + b_proj[None, :]
    return out


# revision 17
# speedup vs baseline: 1.2937x; 1.2171x over previous
"""Trainium2 Bass kernel: causal self-attention with RoPE.

Model (matches the reference nn.Module):
    B=4, T=2048, C=1024, H=16 heads, head_dim=64
    qkv = x @ W_attn + b_attn ; rope(q, k) ; causal softmax(q k^T / 8) @ v
    out = y @ W_proj + b_proj

Sharding over 8 NeuronCores: data parallel on batch (4) x tensor parallel on
heads (2 groups of 8). Each core computes its batch's 8 heads end to end and
a partial y @ W_proj over its 512 head-dims; the host sums the two partial
projections per batch and adds b_proj.

Everything on-chip stays in "feature on partitions" (transposed) layout so
every matmul contracts over the partition dim with zero transposes:
  x^T [C,T] -> K^T [512,T] resident / Q^T per 512-query stripe (RoPE fused
  into the psum eviction via a partition-swapped SBUF->SBUF DMA + two table
  multiplies), V [T,512] natural with a ones column per head (the softmax
  denominator falls out of the same matmul that accumulates att @ V).
  Matmuls run as float32r (TF32-like) at full PE rate.

The program is emitted stripe-interleaved (QKV for 512 t-columns ->
attention for that query stripe -> output projection for those columns) with
every tile pool open for the whole kernel, so the Tile scheduler can overlap
the PE-heavy projection phases with the ACT-heavy softmax phase.
"""

import os
import sys
from contextlib import ExitStack

for _p in ("/opt/trn_rl_repo", "/root/.axon_site/_ro/trn_rl_repo"):
    if os.path.isdir(_p) and _p not in sys.path:
        sys.path.append(_p)

import numpy as np

import bass_rust
import concourse.bass as bass
import concourse.mybir as mybir
from concourse import tile
from concourse.bass_utils import run_bass_kernel_spmd

F32 = mybir.dt.float32
F32R = mybir.dt.float32r
Act = mybir.ActivationFunctionType

B, T, C = 4, 2048, 1024
H, HD = 16, 64
HL = 8          # heads per core
N_CORES = 8
ROPE_BASE = 10000.0

T8 = 256        # t slice width for the qkv phase
QB = 512        # query stripe width
KB = 128        # key block for attention
NKB = T // KB   # 16
NQG = T // QB   # 4


def split_excess_waits(nc, max_waits=1):
    """The walrus build in this container supports only one sync-wait command
    per instruction; hoist extra semaphore waits onto same-engine NoOps
    inserted immediately before the instruction (same engine timeline, so
    semantics are unchanged)."""
    ctr = 0
    for fn in nc.m.functions:
        for blk in fn.blocks:
            new_insts = []
            changed = False
            for inst in blk.instructions:
                si = inst.sync_info
                if si is not None:
                    waits = list(si.on_wait)
                    sem_waits = [w for w in waits if w.sync_type == "semaphore"]
                    other = [w for w in waits if w.sync_type != "semaphore"]
                    budget = max(0, max_waits - len(other))
                    if len(sem_waits) > budget:
                        keep = sem_waits[:budget]
                        extra = sem_waits[budget:]
                        step = max(1, max_waits)
                        for i in range(0, len(extra), step):
                            nop = bass_rust.InstNoOp(
                                name=f"WSPLIT-{ctr}", ins=[], outs=[])
                            ctr += 1
                            nop.engine = inst.engine
                            nop.sync_info = bass_rust.SyncInfo(
                                on_wait=extra[i:i + step], on_update=[])
                            new_insts.append(nop)
                        si.on_wait = other + keep
                        changed = True
                new_insts.append(inst)
            if changed:
                blk.instructions = new_insts


def build_nc(split=True):
    nc = bass.Bass("TRN2", target_bir_lowering=False, debug=False,
                   num_devices=N_CORES)

    xT_d = nc.dram_tensor("xT", [C, T], F32R, kind="ExternalInput")
    wq_d = nc.dram_tensor("wq", [C, 512], F32R, kind="ExternalInput")
    wk_d = nc.dram_tensor("wk", [C, 512], F32R, kind="ExternalInput")
    wv_d = nc.dram_tensor("wv", [C, 512], F32R, kind="ExternalInput")
    wp_d = nc.dram_tensor("wp", [512, C], F32R, kind="ExternalInput")
    bq_d = nc.dram_tensor("bq", [512], F32, kind="ExternalInput")
    bk_d = nc.dram_tensor("bk", [512], F32, kind="ExternalInput")
    bv_d = nc.dram_tensor("bvrep", [128, 512], F32, kind="ExternalInput")
    cos_d = nc.dram_tensor("cos128", [128, T], F32R, kind="ExternalInput")
    sin_d = nc.dram_tensor("sin128", [128, T], F32R, kind="ExternalInput")
    mask_d = nc.dram_tensor("masks", [4, 128, QB], F32R, kind="ExternalInput")
    ones_d = nc.dram_tensor("ones128", [128, 128], F32R, kind="ExternalInput")
    outT_d = nc.dram_tensor("outT", [C, T], F32, kind="ExternalOutput")

    with tile.TileContext(nc) as tc, ExitStack() as ctx:
        const = ctx.enter_context(tc.tile_pool(name="const", bufs=1))
        persist = ctx.enter_context(tc.tile_pool(name="persist", bufs=1))
        wres = ctx.enter_context(tc.tile_pool(name="wres", bufs=1))
        cs_pool = ctx.enter_context(tc.tile_pool(name="cs_pool", bufs=1))
        xt_pool = ctx.enter_context(tc.tile_pool(name="xt_pool", bufs=2))
        qts_pool = ctx.enter_context(tc.tile_pool(name="qts_pool", bufs=2))
        aux_pool = ctx.enter_context(tc.tile_pool(name="aux_pool", bufs=2))
        pt_pool = ctx.enter_context(tc.tile_pool(name="pt_pool", bufs=2))
        misc_pool = ctx.enter_context(tc.tile_pool(name="misc_pool", bufs=2))
        yt_pool = ctx.enter_context(tc.tile_pool(name="yt_pool", bufs=1))
        out_pool = ctx.enter_context(tc.tile_pool(name="out_pool", bufs=2))
        ps_pool = ctx.enter_context(
            tc.tile_pool(name="ps_pool", bufs=2, space="PSUM"))
        o_pool = ctx.enter_context(
            tc.tile_pool(name="o_pool", bufs=1, space="PSUM"))

        # ---- constants / weights: tiles declared up front, DMAs emitted
        # just before first use so early queues prioritize the critical path
        ones_sb = const.tile([128, 128], F32R, tag="ones", name="ones_sb")
        bq_sb = const.tile([128, 4], F32, tag="bq", name="bq_sb")
        bk_sb = const.tile([128, 4], F32, tag="bk", name="bk_sb")
        bv_sb = const.tile([128, 512], F32, tag="bv", name="bv_sb")
        mask_sb = [const.tile([128, QB], F32R, tag=f"mask{r}",
                              name=f"mask{r}") for r in range(4)]

        kt_t = [persist.tile([128, T], F32R, tag=f"kt{i}", name=f"kt{i}")
                for i in range(4)]
        v_sb = persist.tile([128, HL * NKB * 65], F32R, tag="v", name="v_sb")

        wq_sb = wres.tile([128, 8 * 512], F32R, tag="wq", name="wq_sb")
        wk_sb = wres.tile([128, 8 * 512], F32R, tag="wk", name="wk_sb")
        wv_sb = wres.tile([128, 8 * 512], F32R, tag="wv", name="wv_sb")
        wp_sb = wres.tile([128, 4 * C], F32R, tag="wp", name="wp_sb")

        # critical-path first: Q weights, then biases; K weights right after
        for cc in range(8):
            nc.sync.dma_start(wq_sb[:, cc * 512:(cc + 1) * 512],
                              wq_d[cc * 128:(cc + 1) * 128, :])
        nc.sync.dma_start(bq_sb[:], bq_d.rearrange("(m p) -> p m", p=128))

        for g in range(NQG):
            # ------- QKV + RoPE for t columns [g*512, (g+1)*512) -------
            gs, ge = g * QB, (g + 1) * QB
            qts = [qts_pool.tile([128, QB], F32R, tag=f"qts{mc}",
                                 name=f"qts{mc}_{g}") for mc in range(4)]
            cosS = cs_pool.tile([128, QB], F32R, tag="cosS", name=f"cosS{g}")
            sinS = cs_pool.tile([128, QB], F32R, tag="sinS", name=f"sinS{g}")
            for t8l in range(2):
                t8 = 2 * g + t8l
                ts, te = t8 * T8, (t8 + 1) * T8
                xt = xt_pool.tile([128, 8 * T8], F32R, tag="xt",
                                  name=f"xt{t8}")
                xv = xt[:].rearrange("p (cc t) -> p cc t", cc=8)
                for half in range(2):
                    nc.sync.dma_start(
                        xv[:, half * 4:(half + 1) * 4],
                        xT_d[512 * half:512 * (half + 1), ts:te]
                        .rearrange("(cc p) t -> p cc t", p=128))
                if t8l == 0:
                    nc.sync.dma_start(cosS[:], cos_d[:, gs:ge])
                    nc.sync.dma_start(sinS[:], sin_d[:, gs:ge])
                if g == 0 and t8l == 0:
                    for cc in range(8):
                        nc.sync.dma_start(wk_sb[:, cc * 512:(cc + 1) * 512],
                                          wk_d[cc * 128:(cc + 1) * 128, :])
                    nc.sync.dma_start(bk_sb[:],
                                      bk_d.rearrange("(m p) -> p m", p=128))

                for wsb, bias_sb, is_q in ((wq_sb, bq_sb, True),
                                           (wk_sb, bk_sb, False)):
                    for mc in range(4):
                        ps = ps_pool.tile([128, T8], F32, tag="qkv",
                                          name=f"ps{t8}_{mc}_{int(is_q)}")
                        for cc in range(8):
                            nc.tensor.matmul(
                                ps[:],
                                lhsT=wsb[:, cc * 512 + mc * 128:
                                         cc * 512 + (mc + 1) * 128],
                                rhs=xt[:, cc * T8:(cc + 1) * T8],
                                start=(cc == 0), stop=(cc == 7))
                        if is_q:
                            dst = qts[mc][:, t8l * T8:(t8l + 1) * T8]
                        else:
                            dst = kt_t[mc][:, ts:te]
                        nc.vector.tensor_scalar_add(dst, ps[:],
                                                    bias_sb[:, mc:mc + 1])

                if g == 0 and t8l == 0:
                    # V weights / bias / ones columns, first needed here
                    for cc in range(8):
                        nc.sync.dma_start(wv_sb[:, cc * 512:(cc + 1) * 512],
                                          wv_d[cc * 128:(cc + 1) * 128, :])
                    nc.sync.dma_start(ones_sb[:], ones_d[:])
                    nc.sync.dma_start(bv_sb[:], bv_d[:])
                    nc.sync.dma_start(
                        v_sb[:].rearrange("p (blk c) -> p blk c",
                                          c=65)[:, :, 64:65],
                        ones_d[:].rearrange("p (b o) -> p b o", o=1))

                for tbl in range(T8 // 128):
                    tb = t8 * (T8 // 128) + tbl
                    ps = ps_pool.tile([128, 512], F32, tag="qkv",
                                      name=f"psv{t8}_{tbl}")
                    for cc in range(8):
                        nc.tensor.matmul(
                            ps[:],
                            lhsT=xt[:, cc * T8 + tbl * 128:
                                    cc * T8 + tbl * 128 + 128],
                            rhs=wv_sb[:, cc * 512:(cc + 1) * 512],
                            start=(cc == 0), stop=(cc == 7))
                    for h in range(HL):
                        off = (h * NKB + tb) * 65
                        nc.vector.tensor_add(
                            v_sb[:, off:off + 64],
                            ps[:, h * 64:(h + 1) * 64],
                            bv_sb[:, h * 64:(h + 1) * 64])

            # ------- RoPE over the full stripe (Q) / t-range (K) -------
            for is_q in (True, False):
                for mc in range(4):
                    dst = qts[mc][:] if is_q else kt_t[mc][:, gs:ge]
                    aux = aux_pool.tile([128, QB], F32R, tag="aux",
                                        name=f"aux{g}_{mc}_{int(is_q)}")
                    nc.sync.dma_start(aux[0:32, :], dst[32:64, :])
                    nc.sync.dma_start(aux[32:64, :], dst[0:32, :])
                    nc.sync.dma_start(aux[64:96, :], dst[96:128, :])
                    nc.sync.dma_start(aux[96:128, :], dst[64:96, :])
                    nc.vector.tensor_mul(dst, dst, cosS[:])
                    nc.gpsimd.tensor_mul(aux[:], aux[:], sinS[:])
                    nc.vector.tensor_add(dst, dst, aux[:])

            # ------- causal attention for query stripe g -------
            if g == 0:
                for r in range(4):
                    nc.sync.dma_start(mask_sb[r][:], mask_d[r])
            for hp in range(4):
                o_ps = [o_pool.tile([65, 512], F32, tag=f"o{hh}",
                                    name=f"o{hh}_{g}_{hp}")
                        for hh in range(2)]
                nkb = 4 * g + 4
                for kb in range(nkb):
                    s_ps = ps_pool.tile([128, 2 * QB], F32, tag="s",
                                        name=f"s_{g}_{hp}_{kb}")
                    for hh in range(2):
                        nc.tensor.matmul(
                            s_ps[:, hh * QB:(hh + 1) * QB],
                            lhsT=kt_t[hp][hh * 64:(hh + 1) * 64,
                                          kb * KB:(kb + 1) * KB],
                            rhs=qts[hp][hh * 64:(hh + 1) * 64, :],
                            start=True, stop=True,
                            tile_position=(hh * 64, 0))
                    pt = pt_pool.tile([128, 2 * QB], F32R, tag="pt",
                                      name=f"pt_{g}_{hp}_{kb}")
                    nc.scalar.activation(pt[:], s_ps[:], Act.Exp, scale=0.125)
                    if kb >= 4 * g:
                        r = kb - 4 * g
                        nc.vector.tensor_mul(pt[:, 0:QB], pt[:, 0:QB],
                                             mask_sb[r][:])
                        nc.vector.tensor_mul(pt[:, QB:2 * QB],
                                             pt[:, QB:2 * QB], mask_sb[r][:])
                    for hh in range(2):
                        h = hp * 2 + hh
                        off = (h * NKB + kb) * 65
                        nc.tensor.matmul(
                            o_ps[hh][:],
                            lhsT=v_sb[:, off:off + 65],
                            rhs=pt[:, hh * QB:(hh + 1) * QB],
                            start=(kb == 0), stop=(kb == nkb - 1))
                if hp == 0:
                    yts = [yt_pool.tile([128, QB], F32R, tag=f"yt{i}",
                                        name=f"yt{i}_{g}") for i in range(4)]
                for hh in range(2):
                    recip = misc_pool.tile([65, 512], F32R, tag="recip",
                                           name=f"rc_{g}_{hp}_{hh}")
                    with nc.allow_low_precision(
                            reason="fp32r softmax denominators"):
                        nc.vector.reciprocal(recip[64:65, :],
                                             o_ps[hh][64:65, :])
                    o_sb = misc_pool.tile([64, 512], F32, tag="osb",
                                          name=f"ob_{g}_{hp}_{hh}")
                    nc.scalar.copy(o_sb[:], o_ps[hh][0:64, :])
                    # reuse the freed O bank for the reciprocal broadcast
                    b_ps = o_pool.tile([64, 512], F32, tag=f"o{hh}",
                                       name=f"b_{g}_{hp}_{hh}")
                    nc.tensor.matmul(b_ps[:], lhsT=ones_sb[64:65, 0:64],
                                     rhs=recip[64:65, :],
                                     start=True, stop=True)
                    nc.vector.tensor_mul(yts[hp][hh * 64:(hh + 1) * 64, :],
                                         o_sb[:], b_ps[:])

            # ------- output projection for t columns of stripe g -------
            if g == 0:
                for cc in range(4):
                    nc.sync.dma_start(wp_sb[:, cc * C:(cc + 1) * C],
                                      wp_d[cc * 128:(cc + 1) * 128, :])
            for co in range(8):
                ps = ps_pool.tile([128, 512], F32, tag="s",
                                  name=f"pps_{g}_{co}")
                for cc in range(4):
                    nc.tensor.matmul(
                        ps[:],
                        lhsT=wp_sb[:, cc * C + co * 128:
                                   cc * C + (co + 1) * 128],
                        rhs=yts[cc][:],
                        start=(cc == 0), stop=(cc == 3))
                osb = out_pool.tile([128, 512], F32, tag="out",
                                    name=f"out_{g}_{co}")
                nc.scalar.copy(osb[:], ps[:])
                nc.sync.dma_start(
                    outT_d[co * 128:(co + 1) * 128, g * QB:(g + 1) * QB],
                    osb[:])

    if split:
        split_excess_waits(nc)
    return nc


_NC = None


def _get_nc():
    global _NC
    if _NC is None:
        _NC = build_nc()
    return _NC


def _rope_tables_128():
    rot = HD // 2  # 32
    inv_freq = 1.0 / (ROPE_BASE ** (np.arange(0, rot, 2, dtype=np.float32)
                                    / np.float32(rot)))
    pos = np.arange(T, dtype=np.float32)
    freqs = np.outer(pos, inv_freq).astype(np.float32)   # [T, 16]
    emb = np.concatenate([freqs, freqs], axis=-1)        # [T, 32]
    cosT = np.cos(emb).astype(np.float32).T              # [32, T]
    sinT = np.sin(emb).astype(np.float32).T
    cos128 = np.ascontiguousarray(np.tile(cosT, (4, 1)))
    sgn = np.ones((128, 1), np.float32)
    sgn[0:32] = -1.0
    sgn[64:96] = -1.0
    sin128 = np.ascontiguousarray(np.tile(sinT, (4, 1)) * sgn)
    return cos128, sin128


def _masks():
    kp = np.arange(128, dtype=np.int64)[:, None]
    qf = np.arange(QB, dtype=np.int64)[None, :]
    out = np.empty((4, 128, QB), np.float32)
    for r in range(4):
        out[r] = ((r * KB + kp) <= qf).astype(np.float32)
    return out


def _in_maps(x, W_attn, b_attn, W_proj):
    cos128, sin128 = _rope_tables_128()
    masks = _masks()
    ones = np.ones((128, 128), np.float32)
    maps = []
    for c in range(N_CORES):
        b, hg = c // 2, c % 2
        sl = slice(hg * 512, (hg + 1) * 512)
        maps.append({
            "xT": np.ascontiguousarray(x[b].T),
            "wq": np.ascontiguousarray(W_attn[:, 0 * C:1 * C][:, sl]),
            "wk": np.ascontiguousarray(W_attn[:, 1 * C:2 * C][:, sl]),
            "wv": np.ascontiguousarray(W_attn[:, 2 * C:3 * C][:, sl]),
            "wp": np.ascontiguousarray(W_proj[sl, :]),
            "bq": np.ascontiguousarray(b_attn[0 * C:1 * C][sl]),
            "bk": np.ascontiguousarray(b_attn[1 * C:2 * C][sl]),
            "bvrep": np.ascontiguousarray(
                np.broadcast_to(b_attn[2 * C:3 * C][sl], (128, 512))),
            "cos128": cos128,
            "sin128": sin128,
            "masks": masks,
            "ones128": ones,
        })
    return maps


def kernel(x, W_attn, b_attn, W_proj, b_proj):
    x = np.asarray(x, dtype=np.float32)
    W_attn = np.asarray(W_attn, dtype=np.float32)
    b_attn = np.asarray(b_attn, dtype=np.float32)
    W_proj = np.asarray(W_proj, dtype=np.float32)
    b_proj = np.asarray(b_proj, dtype=np.float32)

    nc = _get_nc()
    maps = _in_maps(x, W_attn, b_attn, W_proj)
    res = run_bass_kernel_spmd(nc, maps, list(range(N_CORES)))

    out = np.empty((B, T, C), np.float32)
    for b in range(B):
        acc = res.results[2 * b]["outT"] + res.results[2 * b + 1]["outT"]
        out[b] = acc.T + b_proj[None, :]
    return out


# revision 19
# speedup vs baseline: 1.4345x; 1.1089x over previous
"""Trainium2 Bass kernel: causal self-attention with RoPE.

Model (matches the reference nn.Module):
    B=4, T=2048, C=1024, H=16 heads, head_dim=64
    qkv = x @ W_attn + b_attn ; rope(q, k) ; causal softmax(q k^T / 8) @ v
    out = y @ W_proj + b_proj

Sharding over 8 NeuronCores: data parallel on batch (4) x tensor parallel on
heads (2 groups of 8). Each core computes its batch's 8 heads end to end and
a partial y @ W_proj over its 512 head-dims; the host sums the two partial
projections per batch and adds b_proj.

Everything on-chip stays in "feature on partitions" (transposed) layout so
every matmul contracts over the partition dim with zero transposes:
  x^T [C,T] -> K^T [512,T] resident / Q^T per 512-query stripe (RoPE fused
  into the psum eviction via a partition-swapped SBUF->SBUF DMA + two table
  multiplies), V [T,512] natural with a ones column per head (the softmax
  denominator falls out of the same matmul that accumulates att @ V).
  Matmuls run as float32r (TF32-like) at full PE rate.

The program is emitted stripe-interleaved (QKV for 512 t-columns ->
attention for that query stripe -> output projection for those columns) with
every tile pool open for the whole kernel, so the Tile scheduler can overlap
the PE-heavy projection phases with the ACT-heavy softmax phase.
"""

import os
import sys
from contextlib import ExitStack

for _p in ("/opt/trn_rl_repo", "/root/.axon_site/_ro/trn_rl_repo"):
    if os.path.isdir(_p) and _p not in sys.path:
        sys.path.append(_p)

import numpy as np

import bass_rust
import concourse.bass as bass
import concourse.mybir as mybir
from concourse import tile
from concourse.bass_utils import run_bass_kernel_spmd

F32 = mybir.dt.float32
F32R = mybir.dt.float32r
Act = mybir.ActivationFunctionType

B, T, C = 4, 2048, 1024
H, HD = 16, 64
HL = 8          # heads per core
N_CORES = 8
ROPE_BASE = 10000.0

T8 = 256        # t slice width for the qkv phase
QB = 512        # query stripe width
KB = 128        # key block for attention
NKB = T // KB   # 16
NQG = T // QB   # 4


def split_excess_waits(nc, max_waits=1):
    """The walrus build in this container supports only one sync-wait command
    per instruction; hoist extra semaphore waits onto same-engine NoOps
    inserted immediately before the instruction (same engine timeline, so
    semantics are unchanged)."""
    ctr = 0
    for fn in nc.m.functions:
        for blk in fn.blocks:
            new_insts = []
            changed = False
            for inst in blk.instructions:
                si = inst.sync_info
                if si is not None:
                    waits = list(si.on_wait)
                    sem_waits = [w for w in waits if w.sync_type == "semaphore"]
                    other = [w for w in waits if w.sync_type != "semaphore"]
                    budget = max(0, max_waits - len(other))
                    if len(sem_waits) > budget:
                        keep = sem_waits[:budget]
                        extra = sem_waits[budget:]
                        step = max(1, max_waits)
                        for i in range(0, len(extra), step):
                            nop = bass_rust.InstNoOp(
                                name=f"WSPLIT-{ctr}", ins=[], outs=[])
                            ctr += 1
                            nop.engine = inst.engine
                            nop.sync_info = bass_rust.SyncInfo(
                                on_wait=extra[i:i + step], on_update=[])
                            new_insts.append(nop)
                        si.on_wait = other + keep
                        changed = True
                new_insts.append(inst)
            if changed:
                blk.instructions = new_insts


def build_nc(split=True):
    nc = bass.Bass("TRN2", target_bir_lowering=False, debug=False,
                   num_devices=N_CORES)

    xT_d = nc.dram_tensor("xT", [C, T], F32R, kind="ExternalInput")
    wq_d = nc.dram_tensor("wq", [C, 512], F32R, kind="ExternalInput")
    wk_d = nc.dram_tensor("wk", [C, 512], F32R, kind="ExternalInput")
    wv_d = nc.dram_tensor("wv", [C, 512], F32R, kind="ExternalInput")
    wp_d = nc.dram_tensor("wp", [512, C], F32R, kind="ExternalInput")
    bq_d = nc.dram_tensor("bq", [512], F32, kind="ExternalInput")
    bk_d = nc.dram_tensor("bk", [512], F32, kind="ExternalInput")
    bv_d = nc.dram_tensor("bvrep", [128, 512], F32, kind="ExternalInput")
    cos_d = nc.dram_tensor("cos128", [128, T], F32R, kind="ExternalInput")
    sin_d = nc.dram_tensor("sin128", [128, T], F32R, kind="ExternalInput")
    mask_d = nc.dram_tensor("masks", [4, 128, QB], F32R, kind="ExternalInput")
    ones_d = nc.dram_tensor("ones128", [128, 128], F32R, kind="ExternalInput")
    sperm_d = nc.dram_tensor("sperm", [128, 128], F32R, kind="ExternalInput")
    outT_d = nc.dram_tensor("outT", [C, T], F32, kind="ExternalOutput")

    with tile.TileContext(nc) as tc, ExitStack() as ctx:
        const = ctx.enter_context(tc.tile_pool(name="const", bufs=1))
        persist = ctx.enter_context(tc.tile_pool(name="persist", bufs=1))
        wres = ctx.enter_context(tc.tile_pool(name="wres", bufs=1))
        cs_pool = ctx.enter_context(tc.tile_pool(name="cs_pool", bufs=1))
        xt_pool = ctx.enter_context(tc.tile_pool(name="xt_pool", bufs=2))
        qts_pool = ctx.enter_context(tc.tile_pool(name="qts_pool", bufs=2))
        aux_pool = ctx.enter_context(tc.tile_pool(name="aux_pool", bufs=2))
        pt_pool = ctx.enter_context(tc.tile_pool(name="pt_pool", bufs=2))
        misc_pool = ctx.enter_context(tc.tile_pool(name="misc_pool", bufs=2))
        yt_pool = ctx.enter_context(tc.tile_pool(name="yt_pool", bufs=1))
        out_pool = ctx.enter_context(tc.tile_pool(name="out_pool", bufs=2))
        ps_pool = ctx.enter_context(
            tc.tile_pool(name="ps_pool", bufs=2, space="PSUM"))
        o_pool = ctx.enter_context(
            tc.tile_pool(name="o_pool", bufs=1, space="PSUM"))

        # ---- constants / weights: tiles declared up front, DMAs emitted
        # just before first use so early queues prioritize the critical path
        ones_sb = const.tile([128, 128], F32R, tag="ones", name="ones_sb")
        sperm_sb = const.tile([128, 128], F32R, tag="sperm", name="sperm_sb")
        bq_sb = const.tile([128, 4], F32, tag="bq", name="bq_sb")
        bk_sb = const.tile([128, 4], F32, tag="bk", name="bk_sb")
        bv_sb = const.tile([128, 512], F32, tag="bv", name="bv_sb")
        mask_sb = const.tile([128, 4 * QB], F32R, tag="mask", name="mask_sb")

        kt_t = [persist.tile([128, T], F32R, tag=f"kt{i}", name=f"kt{i}")
                for i in range(4)]
        v_sb = persist.tile([128, HL * NKB * 65], F32R, tag="v", name="v_sb")

        wq_sb = wres.tile([128, 8 * 512], F32R, tag="wq", name="wq_sb")
        wk_sb = wres.tile([128, 8 * 512], F32R, tag="wk", name="wk_sb")
        wv_sb = wres.tile([128, 8 * 512], F32R, tag="wv", name="wv_sb")
        wp_sb = wres.tile([128, 4 * C], F32R, tag="wp", name="wp_sb")

        # critical-path first: Q weights, then biases; K weights right after
        for cc in range(2):
            nc.sync.dma_start(
                wq_sb[:].rearrange("p (cc m) -> p cc m", cc=8)[:, 4 * cc:4 * cc + 4],
                wq_d[512 * cc:512 * (cc + 1), :].rearrange("(cc p) m -> p cc m", p=128))
        nc.sync.dma_start(bq_sb[:], bq_d.rearrange("(m p) -> p m", p=128))
        nc.sync.dma_start(sperm_sb[:], sperm_d[:])

        for g in range(NQG):
            # ------- QKV + RoPE for t columns [g*512, (g+1)*512) -------
            gs, ge = g * QB, (g + 1) * QB
            qts = [qts_pool.tile([128, QB], F32R, tag=f"qts{mc}",
                                 name=f"qts{mc}_{g}") for mc in range(4)]
            cosS = cs_pool.tile([128, QB], F32R, tag="cosS", name=f"cosS{g}")
            sinS = cs_pool.tile([128, QB], F32R, tag="sinS", name=f"sinS{g}")
            for t8l in range(2):
                t8 = 2 * g + t8l
                ts, te = t8 * T8, (t8 + 1) * T8
                xt = xt_pool.tile([128, 8 * T8], F32R, tag="xt",
                                  name=f"xt{t8}")
                xv = xt[:].rearrange("p (cc t) -> p cc t", cc=8)
                for half in range(2):
                    nc.sync.dma_start(
                        xv[:, half * 4:(half + 1) * 4],
                        xT_d[512 * half:512 * (half + 1), ts:te]
                        .rearrange("(cc p) t -> p cc t", p=128))
                if t8l == 0:
                    nc.sync.dma_start(cosS[:], cos_d[:, gs:ge])
                    nc.sync.dma_start(sinS[:], sin_d[:, gs:ge])
                if g == 0 and t8l == 0:
                    for cc in range(2):
                        nc.sync.dma_start(
                            wk_sb[:].rearrange("p (cc m) -> p cc m",
                                               cc=8)[:, 4 * cc:4 * cc + 4],
                            wk_d[512 * cc:512 * (cc + 1), :]
                            .rearrange("(cc p) m -> p cc m", p=128))
                    nc.sync.dma_start(bk_sb[:],
                                      bk_d.rearrange("(m p) -> p m", p=128))

                for wsb, bias_sb, is_q in ((wq_sb, bq_sb, True),
                                           (wk_sb, bk_sb, False)):
                    for mc in range(4):
                        ps = ps_pool.tile([128, T8], F32, tag="qkv",
                                          name=f"ps{t8}_{mc}_{int(is_q)}")
                        for cc in range(8):
                            nc.tensor.matmul(
                                ps[:],
                                lhsT=wsb[:, cc * 512 + mc * 128:
                                         cc * 512 + (mc + 1) * 128],
                                rhs=xt[:, cc * T8:(cc + 1) * T8],
                                start=(cc == 0), stop=(cc == 7))
                        if is_q:
                            dst = qts[mc][:, t8l * T8:(t8l + 1) * T8]
                        else:
                            dst = kt_t[mc][:, ts:te]
                        nc.vector.tensor_scalar_add(dst, ps[:],
                                                    bias_sb[:, mc:mc + 1])

                if g == 0 and t8l == 0:
                    # V weights / bias / ones columns, first needed here
                    for cc in range(2):
                        nc.sync.dma_start(
                            wv_sb[:].rearrange("p (cc m) -> p cc m",
                                               cc=8)[:, 4 * cc:4 * cc + 4],
                            wv_d[512 * cc:512 * (cc + 1), :]
                            .rearrange("(cc p) m -> p cc m", p=128))
                    nc.sync.dma_start(ones_sb[:], ones_d[:])
                    nc.sync.dma_start(bv_sb[:], bv_d[:])
                    nc.sync.dma_start(
                        v_sb[:].rearrange("p (blk c) -> p blk c",
                                          c=65)[:, :, 64:65],
                        ones_d[:].rearrange("p (b o) -> p b o", o=1))

                for tbl in range(T8 // 128):
                    tb = t8 * (T8 // 128) + tbl
                    ps = ps_pool.tile([128, 512], F32, tag="qkv",
                                      name=f"psv{t8}_{tbl}")
                    for cc in range(8):
                        nc.tensor.matmul(
                            ps[:],
                            lhsT=xt[:, cc * T8 + tbl * 128:
                                    cc * T8 + tbl * 128 + 128],
                            rhs=wv_sb[:, cc * 512:(cc + 1) * 512],
                            start=(cc == 0), stop=(cc == 7))
                    nc.vector.tensor_add(
                        v_sb[:].rearrange("p (h t c) -> p h t c",
                                          h=HL, c=65)[:, :, tb, 0:64],
                        ps[:].rearrange("p (h c) -> p h c", h=HL),
                        bv_sb[:].rearrange("p (h c) -> p h c", h=HL))

            # ------- RoPE over the full stripe (Q) / t-range (K):
            # aux = Perm @ dst on the PE, then dst = dst*cos + aux*sin
            for is_q in (True, False):
                for mc in range(4):
                    dst = qts[mc][:] if is_q else kt_t[mc][:, gs:ge]
                    aux_ps = ps_pool.tile([128, QB], F32, tag="qkv",
                                          name=f"auxp{g}_{mc}_{int(is_q)}")
                    nc.tensor.matmul(aux_ps[:], lhsT=sperm_sb[:], rhs=dst,
                                     start=True, stop=True)
                    aux = aux_pool.tile([128, QB], F32, tag="aux",
                                        name=f"aux{g}_{mc}_{int(is_q)}")
                    nc.vector.tensor_mul(aux[:], aux_ps[:], sinS[:])
                    nc.gpsimd.tensor_mul(dst, dst, cosS[:])
                    nc.vector.tensor_add(dst, dst, aux[:])

            # ------- causal attention for query stripe g -------
            if g == 0:
                nc.sync.dma_start(
                    mask_sb[:].rearrange("p (r q) -> p r q", r=4),
                    mask_d.rearrange("r p q -> p r q"))
            for hp in range(4):
                o_ps = [o_pool.tile([65, 512], F32, tag=f"o{hh}",
                                    name=f"o{hh}_{g}_{hp}")
                        for hh in range(2)]
                nkb = 4 * g + 4
                for kb in range(nkb):
                    r = kb - 4 * g if kb >= 4 * g else None
                    qlo = r * KB if r else 0  # r None/0 -> full width
                    s_ps = ps_pool.tile([128, 2 * QB], F32, tag="s",
                                        name=f"s_{g}_{hp}_{kb}")
                    for hh in range(2):
                        nc.tensor.matmul(
                            s_ps[:, hh * QB + qlo:(hh + 1) * QB],
                            lhsT=kt_t[hp][hh * 64:(hh + 1) * 64,
                                          kb * KB:(kb + 1) * KB],
                            rhs=qts[hp][hh * 64:(hh + 1) * 64, qlo:],
                            start=True, stop=True,
                            tile_position=(hh * 64, 0))
                    pt = pt_pool.tile([128, 2 * QB], F32R, tag="pt",
                                      name=f"pt_{g}_{hp}_{kb}")
                    if qlo == 0:
                        nc.scalar.activation(pt[:], s_ps[:], Act.Exp,
                                             scale=0.125)
                    else:
                        for hh in range(2):
                            nc.scalar.activation(
                                pt[:, hh * QB + qlo:(hh + 1) * QB],
                                s_ps[:, hh * QB + qlo:(hh + 1) * QB],
                                Act.Exp, scale=0.125)
                    if r is not None:
                        for hh in range(2):
                            nc.vector.tensor_mul(
                                pt[:, hh * QB + qlo:(hh + 1) * QB],
                                pt[:, hh * QB + qlo:(hh + 1) * QB],
                                mask_sb[:, r * QB + qlo:(r + 1) * QB])
                    for hh in range(2):
                        h = hp * 2 + hh
                        off = (h * NKB + kb) * 65
                        nc.tensor.matmul(
                            o_ps[hh][:, qlo:],
                            lhsT=v_sb[:, off:off + 65],
                            rhs=pt[:, hh * QB + qlo:(hh + 1) * QB],
                            start=(kb == 0), stop=(kb == nkb - 1))
                if hp == 0:
                    yts = [yt_pool.tile([128, QB], F32R, tag=f"yt{i}",
                                        name=f"yt{i}_{g}") for i in range(4)]
                for hh in range(2):
                    recip = misc_pool.tile([65, 512], F32R, tag="recip",
                                           name=f"rc_{g}_{hp}_{hh}")
                    with nc.allow_low_precision(
                            reason="fp32r softmax denominators"):
                        nc.vector.reciprocal(recip[64:65, :],
                                             o_ps[hh][64:65, :])
                    o_sb = misc_pool.tile([64, 512], F32, tag="osb",
                                          name=f"ob_{g}_{hp}_{hh}")
                    nc.scalar.copy(o_sb[:], o_ps[hh][0:64, :])
                    # reuse the freed O bank for the reciprocal broadcast
                    b_ps = o_pool.tile([64, 512], F32, tag=f"o{hh}",
                                       name=f"b_{g}_{hp}_{hh}")
                    nc.tensor.matmul(b_ps[:], lhsT=ones_sb[64:65, 0:64],
                                     rhs=recip[64:65, :],
                                     start=True, stop=True)
                    nc.vector.tensor_mul(yts[hp][hh * 64:(hh + 1) * 64, :],
                                         o_sb[:], b_ps[:])

            # ------- output projection for t columns of stripe g -------
            if g == 0:
                for cc in range(4):
                    nc.sync.dma_start(wp_sb[:, cc * C:(cc + 1) * C],
                                      wp_d[cc * 128:(cc + 1) * 128, :])
            for co in range(8):
                ps = ps_pool.tile([128, 512], F32, tag="s",
                                  name=f"pps_{g}_{co}")
                for cc in range(4):
                    nc.tensor.matmul(
                        ps[:],
                        lhsT=wp_sb[:, cc * C + co * 128:
                                   cc * C + (co + 1) * 128],
                        rhs=yts[cc][:],
                        start=(cc == 0), stop=(cc == 3))
                osb = out_pool.tile([128, 512], F32, tag="out",
                                    name=f"out_{g}_{co}")
                nc.scalar.copy(osb[:], ps[:])
                nc.sync.dma_start(
                    outT_d[co * 128:(co + 1) * 128, g * QB:(g + 1) * QB],
                    osb[:])

    if split:
        split_excess_waits(nc)
    return nc


_NC = None


def _get_nc():
    global _NC
    if _NC is None:
        _NC = build_nc()
    return _NC


def _rope_tables_128():
    rot = HD // 2  # 32
    inv_freq = 1.0 / (ROPE_BASE ** (np.arange(0, rot, 2, dtype=np.float32)
                                    / np.float32(rot)))
    pos = np.arange(T, dtype=np.float32)
    freqs = np.outer(pos, inv_freq).astype(np.float32)   # [T, 16]
    emb = np.concatenate([freqs, freqs], axis=-1)        # [T, 32]
    cosT = np.cos(emb).astype(np.float32).T              # [32, T]
    sinT = np.sin(emb).astype(np.float32).T
    cos128 = np.ascontiguousarray(np.tile(cosT, (4, 1)))
    sgn = np.ones((128, 1), np.float32)
    sgn[0:32] = -1.0
    sgn[64:96] = -1.0
    sin128 = np.ascontiguousarray(np.tile(sinT, (4, 1)) * sgn)
    return cos128, sin128


def _sperm():
    # permutation: aux[m] = dst[swap(m)], swap exchanges 32-halves in each
    # 64-row head block (sign handled by the sin table)
    P = np.zeros((128, 128), np.float32)
    for m in range(128):
        blk, r = m // 64, m % 64
        k = blk * 64 + (r + 32) % 64
        P[k, m] = 1.0
    return P


def _masks():
    kp = np.arange(128, dtype=np.int64)[:, None]
    qf = np.arange(QB, dtype=np.int64)[None, :]
    out = np.empty((4, 128, QB), np.float32)
    for r in range(4):
        out[r] = ((r * KB + kp) <= qf).astype(np.float32)
    return out


def _in_maps(x, W_attn, b_attn, W_proj):
    cos128, sin128 = _rope_tables_128()
    masks = _masks()
    ones = np.ones((128, 128), np.float32)
    sperm = _sperm()
    maps = []
    for c in range(N_CORES):
        b, hg = c // 2, c % 2
        sl = slice(hg * 512, (hg + 1) * 512)
        maps.append({
            "xT": np.ascontiguousarray(x[b].T),
            "wq": np.ascontiguousarray(W_attn[:, 0 * C:1 * C][:, sl]),
            "wk": np.ascontiguousarray(W_attn[:, 1 * C:2 * C][:, sl]),
            "wv": np.ascontiguousarray(W_attn[:, 2 * C:3 * C][:, sl]),
            "wp": np.ascontiguousarray(W_proj[sl, :]),
            "bq": np.ascontiguousarray(b_attn[0 * C:1 * C][sl]),
            "bk": np.ascontiguousarray(b_attn[1 * C:2 * C][sl]),
            "bvrep": np.ascontiguousarray(
                np.broadcast_to(b_attn[2 * C:3 * C][sl], (128, 512))),
            "sperm": sperm,
            "cos128": cos128,
            "sin128": sin128,
            "masks": masks,
            "ones128": ones,
        })
    return maps


def kernel(x, W_attn, b_attn, W_proj, b_proj):
    x = np.asarray(x, dtype=np.float32)
    W_attn = np.asarray(W_attn, dtype=np.float32)
    b_attn = np.asarray(b_attn, dtype=np.float32)
    W_proj = np.asarray(W_proj, dtype=np.float32)
    b_proj = np.asarray(b_proj, dtype=np.float32)

    nc = _get_nc()
    maps = _in_maps(x, W_attn, b_attn, W_proj)
    res = run_bass_kernel_spmd(nc, maps, list(range(N_CORES)))

    out = np.empty((B, T, C), np.float32)
    for b in range(B):
        acc = res.results[2 * b]["outT"] + res.results[2 * b + 1]["outT"]
        out[b] = acc.T + b_proj[None, :]
    return out
